# revision 18
# baseline (speedup 1.0000x reference)
"""Trainium2 Bass kernel for a dense transformer decoder layer.

Model: B=2, S=2048, H=2048, NH=16, HD=128, FF=8192, fp32 I/O.

Sharding (8 NeuronCores): DP-2 over batch x seq-DP-2 (even/odd token
interleave) across HBM-pairs x TP-2 over heads / FF inside each HBM pair.

  core c: pair p=c//2, head-half hh=c%2; batch b=p//2, parity par=p%2.
  The pair handles the 1024 tokens of batch b at positions par::2.
  Each core owns 8 heads (column half of wq/wk/wv, row half of wo) and
  half of FF.  K/V for all 2048 batch tokens are computed locally
  (replicated inside the batch), so the only cross-core traffic is the
  o_proj / down_proj partial-sum exchange between the two cores of an
  HBM pair, through pair-shared DRAM, with a tiny 2-rank collective
  AllReduce as the barrier.

All activations flow transposed (features on partitions, tokens on the
free axis), so every matmul takes its operands in natural layout and no
on-chip transposes are needed.  Matmuls run in bf16 with fp32 PSUM
accumulation; the residual stream, partial sums and softmax statistics
stay fp32.  RMSNorm variances and softmax denominators are partition-dim
reductions done on the PE with a ones vector.
"""

import sys

sys.path.insert(0, "/opt/trn_rl_repo")

import contextlib

import numpy as np

import concourse.bass as bass
import concourse.tile as tile
from concourse import bacc, mybir
from concourse.bass_utils import run_bass_kernel_spmd

dt = mybir.dt

B, S, H = 2, 2048, 2048
NH, HD = 16, 128
FF = 8192
EPS = 1e-6
N_CORES = 8

TOK = S // 2          # own tokens per pair (1024)
HH = H // 2           # per-core head columns (1024)
FFH = FF // 2         # per-core FF (4096)
NHT = H // 128        # 16
NFT = FFH // 128      # 32
SCALE = 1.0 / float(np.sqrt(HD))
PAIRS = [[0, 1], [2, 3], [4, 5], [6, 7]]


def _rt(ap):
    """[T*128, C] -> [128, T, C] (tile index as middle axis)."""
    return ap.rearrange("(t p) c -> p t c", p=128)


def build_nc():
    nc = bacc.Bacc(None, num_devices=N_CORES)

    # ---------------- I/O ----------------
    xt_e = nc.dram_tensor("xt", [H, S], dt.float32, kind="ExternalInput")
    xto_e = nc.dram_tensor("xt_own", [H, TOK], dt.float32, kind="ExternalInput")
    mk_e = nc.dram_tensor("maskt", [S, TOK], dt.float32, kind="ExternalInput")
    wq_e = nc.dram_tensor("wq", [H, HH], dt.bfloat16, kind="ExternalInput")
    wk_e = nc.dram_tensor("wk", [H, HH], dt.bfloat16, kind="ExternalInput")
    wv_e = nc.dram_tensor("wv", [H, HH], dt.bfloat16, kind="ExternalInput")
    wo_e = nc.dram_tensor("wo", [HH, H], dt.bfloat16, kind="ExternalInput")
    wg_e = nc.dram_tensor("wg", [H, FFH], dt.bfloat16, kind="ExternalInput")
    wu_e = nc.dram_tensor("wu", [H, FFH], dt.bfloat16, kind="ExternalInput")
    wd_e = nc.dram_tensor("wd", [FFH, H], dt.bfloat16, kind="ExternalInput")
    gi_e = nc.dram_tensor("g_in", [H, 1], dt.float32, kind="ExternalInput")
    gp_e = nc.dram_tensor("g_post", [H, 1], dt.float32, kind="ExternalInput")
    slot_e = nc.dram_tensor("slot", [1, 2], dt.uint32, kind="ExternalInput")
    out_e = nc.dram_tensor("out", [H, TOK], dt.float32, kind="ExternalOutput")

    # ---------------- internal DRAM ----------------
    kt_d = nc.dram_tensor("kt_d", [HH, S], dt.bfloat16)       # K^T
    v_d = nc.dram_tensor("v_d", [S, HH], dt.bfloat16)         # V natural
    qt_d = nc.dram_tensor("qt_d", [HH, TOK], dt.bfloat16)     # Q^T
    at_d = nc.dram_tensor("at_d", [HH, TOK], dt.bfloat16)     # attn^T
    x2_d = nc.dram_tensor("x2_d", [H, TOK], dt.float32)       # residual after attn
    bb_d = nc.dram_tensor("bb_d", [1, S], dt.float32)         # rstd bounce
    bb2_d = nc.dram_tensor("bb2_d", [1, TOK], dt.float32)
    bbq_d = nc.dram_tensor("bbq_d", [1, TOK], dt.float32)
    bbd_d = nc.dram_tensor("bbd_d", [16, 512], dt.float32)    # attn denom bounce
    xo_d = nc.dram_tensor("xo_d", [2, 128, NHT * TOK], dt.float32, addr_space="Shared")
    xd_d = nc.dram_tensor("xd_d", [2, 128, NHT * TOK], dt.float32, addr_space="Shared")
    b1i_d = nc.dram_tensor("b1i_d", [128, 1], dt.float32)
    b1o_d = nc.dram_tensor("b1o_d", [128, 1], dt.float32)
    b2i_d = nc.dram_tensor("b2i_d", [128, 1], dt.float32)
    b2o_d = nc.dram_tensor("b2o_d", [128, 1], dt.float32)
    b3i_d = nc.dram_tensor("b3i_d", [128, 1], dt.float32)
    b3o_d = nc.dram_tensor("b3o_d", [128, 1], dt.float32)
    b4i_d = nc.dram_tensor("b4i_d", [128, 1], dt.float32)
    b4o_d = nc.dram_tensor("b4o_d", [128, 1], dt.float32)

    xt_t = _rt(xt_e[:])
    xto_t = _rt(xto_e[:])
    mk_t = _rt(mk_e[:])
    wq_t = _rt(wq_e[:])
    wo_t = _rt(wo_e[:])
    wg_t = _rt(wg_e[:])
    wu_t = _rt(wu_e[:])
    wd_t = _rt(wd_e[:])
    gi_t = _rt(gi_e[:])
    gp_t = _rt(gp_e[:])
    kt_dt = _rt(kt_d[:])
    v_dt = _rt(v_d[:])
    qt_dt = _rt(qt_d[:])
    at_dt = _rt(at_d[:])
    x2_dt = _rt(x2_d[:])
    out_t = _rt(out_e[:])

    Exp = mybir.ActivationFunctionType.Exp
    Silu = mybir.ActivationFunctionType.Silu
    Sqrt = mybir.ActivationFunctionType.Sqrt
    MUL = mybir.AluOpType.mult

    def bcast_ap(dram_t, offset, width):
        return bass.AP(tensor=dram_t, offset=offset, ap=[[0, 128], [1, width]])

    with tile.TileContext(nc) as tc, contextlib.ExitStack() as top:
        glob = top.enter_context(tc.tile_pool(name="glob", bufs=1))
        r = nc.sync.alloc_register("slotr")
        nc.sync.reg_load(r, slot_e[0:1, 0:1])
        off = nc.sync.snap(r, donate=True, min_val=0, max_val=1)

        ones_r = glob.tile([128, 1], dt.float32r)
        ones_b = glob.tile([128, 1], dt.bfloat16)
        tmp1 = glob.tile([128, 1], dt.float32)
        nc.vector.memset(tmp1[:], 1.0)
        nc.vector.tensor_copy(ones_r[:], tmp1[:])
        nc.vector.tensor_copy(ones_b[:], tmp1[:])
        eps1 = glob.tile([1, 1], dt.float32)
        nc.vector.memset(eps1[:], EPS)
        gi_sb = glob.tile([128, NHT], dt.float32)
        gp_sb = glob.tile([128, NHT], dt.float32)
        nc.sync.dma_start(out=gi_sb[:], in_=gi_t[:, :, 0])
        nc.sync.dma_start(out=gp_sb[:], in_=gp_t[:, :, 0])

        # ============ Phase 1: rmsnorm(x) -> h; K^T and V for all 2048
        # ============ batch tokens (own 8 heads)
        CH = 256
        NCH = S // CH
        with contextlib.ExitStack() as ph:
            wkv = ph.enter_context(tc.tile_pool(name="wkv", bufs=1))
            xin = ph.enter_context(tc.tile_pool(name="xin", bufs=2))
            hpool = ph.enter_context(tc.tile_pool(name="hpool", bufs=2))
            sm1 = ph.enter_context(tc.tile_pool(name="sm1", bufs=3))
            sqp = ph.enter_context(tc.tile_pool(name="sqp", bufs=3))
            kvo = ph.enter_context(tc.tile_pool(name="kvo", bufs=4))
            psv = ph.enter_context(tc.tile_pool(name="psv", bufs=2, space="PSUM"))
            psk = ph.enter_context(tc.tile_pool(name="psk", bufs=3, space="PSUM"))

            wk_sb = wkv.tile([128, NHT, HH], dt.bfloat16)
            wv_sb = wkv.tile([128, NHT, HH], dt.bfloat16)

            for ci in range(NCH):
                sl = slice(ci * CH, (ci + 1) * CH)
                x_sb = xin.tile([128, NHT, CH], dt.float32)
                nc.sync.dma_start(out=x_sb[:], in_=xt_t[:, :, sl])
                if ci == 0:
                    # big weight loads on the otherwise-idle SWDGE queue so
                    # they don't block the x-chunk loads on the sync queue
                    nc.gpsimd.dma_start(out=wk_sb[:], in_=_rt(wk_e[:]))
                    nc.gpsimd.dma_start(out=wv_sb[:], in_=_rt(wv_e[:]))
                pvar = psv.tile([1, CH], dt.float32)
                for ht in range(NHT):
                    sq = sqp.tile([128, CH], dt.float32r)
                    nc.vector.tensor_mul(sq[:], x_sb[:, ht, :], x_sb[:, ht, :])
                    nc.tensor.matmul(pvar[:], ones_r[:], sq[:],
                                     start=(ht == 0), stop=(ht == NHT - 1))
                std = sm1.tile([1, CH], dt.float32)
                nc.scalar.activation(std[:], pvar[:], Sqrt, scale=1.0 / H, bias=eps1[:])
                rstd = sm1.tile([1, CH], dt.float32)
                nc.vector.reciprocal(rstd[:], std[:])
                nc.sync.dma_start(out=bb_d[0:1, sl], in_=rstd[:])
                bc = sm1.tile([128, CH], dt.float32)
                nc.sync.dma_start(out=bc[:], in_=bcast_ap(bb_d, ci * CH, CH))
                h_sb = hpool.tile([128, NHT, CH], dt.bfloat16)
                for ht in range(NHT):
                    nc.vector.scalar_tensor_tensor(
                        h_sb[:, ht, :], x_sb[:, ht, :], gi_sb[:, ht:ht + 1], bc[:],
                        MUL, MUL)
                # K^T tiles [kcol 128, CH]
                for kc in range(HH // 128):
                    pk = psk.tile([128, CH], dt.float32)
                    for ht in range(NHT):
                        nc.tensor.matmul(pk[:], wk_sb[:, ht, kc * 128:(kc + 1) * 128],
                                         h_sb[:, ht, :],
                                         start=(ht == 0), stop=(ht == NHT - 1))
                    kt_sb = kvo.tile([128, CH], dt.bfloat16)
                    nc.vector.tensor_copy(kt_sb[:], pk[:])
                    nc.sync.dma_start(out=kt_dt[:, kc, sl], in_=kt_sb[:])
                # V tiles [tok 128, 512]
                for tb in range(CH // 128):
                    for vc in range(HH // 512):
                        pv = psk.tile([128, 512], dt.float32)
                        for ht in range(NHT):
                            nc.tensor.matmul(
                                pv[:], h_sb[:, ht, tb * 128:(tb + 1) * 128],
                                wv_sb[:, ht, vc * 512:(vc + 1) * 512],
                                start=(ht == 0), stop=(ht == NHT - 1))
                        v_sb = kvo.tile([128, 512], dt.bfloat16)
                        nc.vector.tensor_copy(v_sb[:], pv[:])
                        nc.sync.dma_start(
                            out=v_dt[:, ci * (CH // 128) + tb, vc * 512:(vc + 1) * 512],
                            in_=v_sb[:])

        # ============ Phase 1b: rmsnorm(x_own) -> h_own; Q^T over own tokens
        with contextlib.ExitStack() as ph:
            xin = ph.enter_context(tc.tile_pool(name="xin2", bufs=2))
            hop = ph.enter_context(tc.tile_pool(name="hop", bufs=1))
            sm2 = ph.enter_context(tc.tile_pool(name="sm2", bufs=3))
            sqp = ph.enter_context(tc.tile_pool(name="sqp2", bufs=3))
            wqp = ph.enter_context(tc.tile_pool(name="wqp", bufs=1))
            qto = ph.enter_context(tc.tile_pool(name="qto", bufs=4))
            psv = ph.enter_context(tc.tile_pool(name="psv2", bufs=2, space="PSUM"))
            psq = ph.enter_context(tc.tile_pool(name="psq", bufs=2, space="PSUM"))

            h_own = hop.tile([128, NHT, TOK], dt.bfloat16)
            wq_sb = wqp.tile([128, NHT, HH], dt.bfloat16)
            nc.sync.dma_start(out=wq_sb[:], in_=_rt(wq_e[:]))
            for oc2 in range(TOK // 512):
                sl = slice(oc2 * 512, (oc2 + 1) * 512)
                x_sb = xin.tile([128, NHT, 512], dt.float32)
                nc.sync.dma_start(out=x_sb[:], in_=xto_t[:, :, sl])
                pvar = psv.tile([1, 512], dt.float32)
                for ht in range(NHT):
                    sq = sqp.tile([128, 512], dt.float32r)
                    nc.vector.tensor_mul(sq[:], x_sb[:, ht, :], x_sb[:, ht, :])
                    nc.tensor.matmul(pvar[:], ones_r[:], sq[:],
                                     start=(ht == 0), stop=(ht == NHT - 1))
                std = sm2.tile([1, 512], dt.float32)
                nc.scalar.activation(std[:], pvar[:], Sqrt, scale=1.0 / H, bias=eps1[:])
                rstd = sm2.tile([1, 512], dt.float32)
                nc.vector.reciprocal(rstd[:], std[:])
                nc.sync.dma_start(out=bbq_d[0:1, sl], in_=rstd[:])
                bc = sm2.tile([128, 512], dt.float32)
                nc.sync.dma_start(out=bc[:], in_=bcast_ap(bbq_d, oc2 * 512, 512))
                for ht in range(NHT):
                    nc.vector.scalar_tensor_tensor(
                        h_own[:, ht, sl], x_sb[:, ht, :], gi_sb[:, ht:ht + 1], bc[:],
                        MUL, MUL)
                for qc in range(HH // 128):
                    pq = psq.tile([128, 512], dt.float32)
                    for ht in range(NHT):
                        nc.tensor.matmul(pq[:], wq_sb[:, ht, qc * 128:(qc + 1) * 128],
                                         h_own[:, ht, sl],
                                         start=(ht == 0), stop=(ht == NHT - 1))
                    qt_sb = qto.tile([128, 512], dt.bfloat16)
                    nc.vector.tensor_copy(qt_sb[:], pq[:])
                    nc.sync.dma_start(out=qt_dt[:, qc, sl], in_=qt_sb[:])

        # ============ Phase 2: attention (causal over interleaved halves)
        v_re = v_d[:].rearrange("(kb p) c -> p kb c", p=128)
        mk_re = mk_e[:].rearrange("(kb p) q -> p kb q", p=128)
        ph23 = contextlib.ExitStack()
        atp0 = ph23.enter_context(tc.tile_pool(name="atp0", bufs=1))
        at23 = atp0.tile([128, 8, TOK], dt.bfloat16)
        wo_sb = atp0.tile([128, 8, H], dt.bfloat16)
        with contextlib.ExitStack() as ph:
            qrow_p = ph.enter_context(tc.tile_pool(name="qrow", bufs=2))
            mskp = ph.enter_context(tc.tile_pool(name="mskp", bufs=2))
            kvp = ph.enter_context(tc.tile_pool(name="kvp", bufs=3))
            expp = ph.enter_context(tc.tile_pool(name="expp", bufs=2))
            esp = ph.enter_context(tc.tile_pool(name="esp", bufs=4))
            smd = ph.enter_context(tc.tile_pool(name="smd", bufs=3))
            ato = ph.enter_context(tc.tile_pool(name="ato", bufs=3))
            pss = ph.enter_context(tc.tile_pool(name="pss", bufs=4, space="PSUM"))
            psd = ph.enter_context(tc.tile_pool(name="psd", bufs=2, space="PSUM"))
            psu = ph.enter_context(tc.tile_pool(name="psu", bufs=2, space="PSUM"))

            for oc2 in range(TOK // 512):
                qsl = slice(oc2 * 512, (oc2 + 1) * 512)
                if oc2 == 1:
                    nc.sync.dma_start(out=wo_sb[:], in_=_rt(wo_e[:]))
                nkb = 8 * (oc2 + 1)
                kext = nkb * 128
                msk = mskp.tile([128, nkb, 512], dt.float32, tag="msk")
                nc.sync.dma_start(out=msk[:], in_=mk_re[:, 0:nkb, qsl])
                qrow = qrow_p.tile([128, 8, 512], dt.bfloat16)
                nc.sync.dma_start(out=qrow[:], in_=qt_dt[:, :, qsl])
                for h in range(8):
                    kth = kvp.tile([128, nkb * 128], dt.bfloat16, tag="kth")
                    nc.sync.dma_start(out=kth[:], in_=kt_dt[:, h, 0:kext])
                    vth = kvp.tile([128, nkb, 128], dt.bfloat16, tag="vth")
                    nc.sync.dma_start(out=vth[:],
                                      in_=v_re[:, 0:nkb, h * 128:(h + 1) * 128])
                    exps = expp.tile([128, nkb, 512], dt.bfloat16, tag="exps")
                    for kb in range(nkb):
                        ps = pss.tile([128, 512], dt.float32)
                        nc.tensor.matmul(ps[:], kth[:, kb * 128:(kb + 1) * 128],
                                         qrow[:, h, :], start=True, stop=True)
                        es = esp.tile([128, 512], dt.float32)
                        nc.vector.scalar_tensor_tensor(
                            es[:], ps[:], SCALE, msk[:, kb, :], MUL,
                            mybir.AluOpType.add)
                        nc.scalar.activation(exps[:, kb, :], es[:], Exp)
                    pd = psd.tile([1, 512], dt.float32)
                    for kb in range(nkb):
                        nc.tensor.matmul(pd[:], ones_b[:], exps[:, kb, :],
                                         start=(kb == 0), stop=(kb == nkb - 1))
                    dd = smd.tile([1, 512], dt.float32)
                    nc.vector.reciprocal(dd[:], pd[:])
                    nc.sync.dma_start(out=bbd_d[oc2 * 8 + h:oc2 * 8 + h + 1, :],
                                      in_=dd[:])
                    bcd = smd.tile([128, 512], dt.float32)
                    nc.sync.dma_start(out=bcd[:],
                                      in_=bcast_ap(bbd_d, (oc2 * 8 + h) * 512, 512))
                    pu = psu.tile([128, 512], dt.float32)
                    for kb in range(nkb):
                        nc.tensor.matmul(pu[:], vth[:, kb, :], exps[:, kb, :],
                                         start=(kb == 0), stop=(kb == nkb - 1))
                    nc.vector.tensor_tensor(at23[:, h, qsl], pu[:], bcd[:], MUL)

        # ============ Phase 3: o_proj partial, pair exchange, x2 residual
        with contextlib.ExitStack() as ph:
            otp = ph.enter_context(tc.tile_pool(name="otp", bufs=3))
            rxp = ph.enter_context(tc.tile_pool(name="rxp", bufs=4))
            pso = ph.enter_context(tc.tile_pool(name="pso", bufs=4, space="PSUM"))
            psv3 = ph.enter_context(tc.tile_pool(name="psv3", bufs=2, space="PSUM"))
            sq3p = ph.enter_context(tc.tile_pool(name="sq3p", bufs=3))
            sm3 = ph.enter_context(tc.tile_pool(name="sm3", bufs=2))

            owrites = []
            for ocl in range(NHT):
                o_t = otp.tile([128, TOK], dt.float32)
                for oc2 in range(TOK // 512):
                    po = pso.tile([128, 512], dt.float32)
                    for hdt in range(8):
                        nc.tensor.matmul(po[:], wo_sb[:, hdt, ocl * 128:(ocl + 1) * 128],
                                         at23[:, hdt, oc2 * 512:(oc2 + 1) * 512],
                                         start=(hdt == 0), stop=(hdt == 7))
                    nc.vector.tensor_copy(o_t[:, oc2 * 512:(oc2 + 1) * 512], po[:])
                d = nc.sync.dma_start(
                    out=xo_d[bass.ds(off, 1), :, ocl * TOK:(ocl + 1) * TOK],
                    in_=o_t[:])
                owrites.append(d)

            # barrier 1, split in halves: the first barrier overlaps the
            # second half of the o_proj matmuls
            b1 = rxp.tile([128, 1], dt.float32)
            nc.vector.memset(b1[:], 1.0)
            nc.sync.dma_start(out=b1i_d[:], in_=b1[:])
            cc1a = nc.gpsimd.collective_compute(
                "AllReduce", mybir.AluOpType.add, replica_groups=PAIRS,
                ins=[b1i_d[:].opt()], outs=[b1o_d[:].opt()])
            for d in owrites[:NHT // 2]:
                tile.add_dep_helper(cc1a.ins, d.ins, sync=True, reason="o writes before barrier")
            nc.sync.dma_start(out=b2i_d[:], in_=b1[:])
            cc1b = nc.gpsimd.collective_compute(
                "AllReduce", mybir.AluOpType.add, replica_groups=PAIRS,
                ins=[b2i_d[:].opt()], outs=[b2o_d[:].opt()])
            for d in owrites[NHT // 2:]:
                tile.add_dep_helper(cc1b.ins, d.ins, sync=True, reason="o writes before barrier")

            pvar30 = psv3.tile([1, 512], dt.float32, tag="pvar3")
            pvar31 = psv3.tile([1, 512], dt.float32, tag="pvar3")
            pvars3 = [pvar30, pvar31]
            for ocl in range(NHT):
                tsl = slice(ocl * TOK, (ocl + 1) * TOK)
                oa = rxp.tile([128, TOK], dt.float32, tag="oa")
                ob = rxp.tile([128, TOK], dt.float32, tag="ob")
                cc1 = cc1a if ocl < NHT // 2 else cc1b
                da = nc.sync.dma_start(out=oa[:], in_=xo_d[0, :, tsl])
                db = nc.sync.dma_start(out=ob[:], in_=xo_d[1, :, tsl])
                tile.add_dep_helper(da.ins, cc1.ins, sync=True, reason="read after barrier1")
                tile.add_dep_helper(db.ins, cc1.ins, sync=True, reason="read after barrier1")
                xo_sb = rxp.tile([128, TOK], dt.float32, tag="xo")
                nc.sync.dma_start(out=xo_sb[:], in_=xto_t[:, ocl, :])
                x2_t = rxp.tile([128, TOK], dt.float32, tag="x2")
                nc.vector.tensor_add(x2_t[:], oa[:], ob[:])
                nc.vector.tensor_add(x2_t[:], x2_t[:], xo_sb[:])
                nc.sync.dma_start(out=x2_dt[:, ocl, :], in_=x2_t[:])
                for oc2 in range(TOK // 512):
                    sl2 = slice(oc2 * 512, (oc2 + 1) * 512)
                    sq3 = sq3p.tile([128, 512], dt.float32r)
                    nc.vector.tensor_mul(sq3[:], x2_t[:, sl2], x2_t[:, sl2])
                    nc.tensor.matmul(pvars3[oc2], ones_r[:], sq3[:],
                                     start=(ocl == 0), stop=(ocl == NHT - 1))
            for oc2 in range(TOK // 512):
                sl2 = slice(oc2 * 512, (oc2 + 1) * 512)
                std3 = sm3.tile([1, 512], dt.float32, tag="std3")
                nc.scalar.activation(std3[:], pvars3[oc2], Sqrt, scale=1.0 / H, bias=eps1[:])
                rstd3 = sm3.tile([1, 512], dt.float32, tag="rstd3")
                nc.vector.reciprocal(rstd3[:], std3[:])
                nc.sync.dma_start(out=bb2_d[0:1, sl2], in_=rstd3[:])

        ph23.close()

        # ============ Phase 4: rmsnorm2 + SwiGLU MLP, down exchange
        with contextlib.ExitStack() as ph:
            h2p = ph.enter_context(tc.tile_pool(name="h2p", bufs=1))
            atp2 = ph.enter_context(tc.tile_pool(name="aTp", bufs=1))
            xz2 = ph.enter_context(tc.tile_pool(name="xz2", bufs=2))
            sm4 = ph.enter_context(tc.tile_pool(name="sm4", bufs=3))
            sqp = ph.enter_context(tc.tile_pool(name="sqp4", bufs=3))
            wgp = ph.enter_context(tc.tile_pool(name="wgp", bufs=2))
            sgp = ph.enter_context(tc.tile_pool(name="sgp", bufs=3))
            dnp = ph.enter_context(tc.tile_pool(name="dnp", bufs=3))
            wdp = ph.enter_context(tc.tile_pool(name="wdp", bufs=2))
            bc2s = []
            for oc2 in range(TOK // 512):
                bc2 = sm4.tile([128, 512], dt.float32, tag="bc4")
                nc.sync.dma_start(out=bc2[:], in_=bcast_ap(bb2_d, oc2 * 512, 512))
                bc2s.append(bc2)
            psg = ph.enter_context(tc.tile_pool(name="psg", bufs=3, space="PSUM"))
            psn = ph.enter_context(tc.tile_pool(name="psn", bufs=2, space="PSUM"))
            h2 = h2p.tile([128, NHT, TOK], dt.bfloat16)
            for ocl in range(NHT):
                xz = xz2.tile([128, TOK], dt.float32, tag="xz")
                nc.sync.dma_start(out=xz[:], in_=x2_dt[:, ocl, :])
                for oc2 in range(TOK // 512):
                    sl = slice(oc2 * 512, (oc2 + 1) * 512)
                    nc.vector.scalar_tensor_tensor(
                        h2[:, ocl, sl], xz[:, sl], gp_sb[:, ocl:ocl + 1], bc2s[oc2],
                        MUL, MUL)

            # gate/up -> aT
            aT = atp2.tile([128, NFT, TOK], dt.bfloat16)
            for ff in range(NFT):
                pg0 = psg.tile([128, 512], dt.float32, tag="pg")
                pg1 = psg.tile([128, 512], dt.float32, tag="pg")
                pu0 = psg.tile([128, 512], dt.float32, tag="pu")
                pu1 = psg.tile([128, 512], dt.float32, tag="pu")
                pgs, pus = [pg0, pg1], [pu0, pu1]
                wg_sb = wgp.tile([128, NHT, 128], dt.bfloat16, tag="wg")
                nc.sync.dma_start(out=wg_sb[:], in_=wg_t[:, :, ff * 128:(ff + 1) * 128])
                wu_sb = wgp.tile([128, NHT, 128], dt.bfloat16, tag="wu")
                nc.sync.dma_start(out=wu_sb[:], in_=wu_t[:, :, ff * 128:(ff + 1) * 128])
                for ht in range(NHT):
                    for oc2 in range(TOK // 512):
                        sl = slice(oc2 * 512, (oc2 + 1) * 512)
                        nc.tensor.matmul(pgs[oc2][:], wg_sb[:, ht, :], h2[:, ht, sl],
                                         start=(ht == 0), stop=(ht == NHT - 1))
                        nc.tensor.matmul(pus[oc2][:], wu_sb[:, ht, :], h2[:, ht, sl],
                                         start=(ht == 0), stop=(ht == NHT - 1))
                for oc2 in range(TOK // 512):
                    sl = slice(oc2 * 512, (oc2 + 1) * 512)
                    sg = sgp.tile([128, 512], dt.float32)
                    nc.scalar.activation(sg[:], pgs[oc2][:], Silu)
                    nc.vector.tensor_tensor(aT[:, ff, sl], sg[:], pus[oc2][:], MUL)

            # down partials + exchange
            dwrites = []
            for hc in range(NHT):
                dn_t = dnp.tile([128, TOK], dt.float32)
                wd_sb = wdp.tile([128, NFT, 128], dt.bfloat16)
                nc.sync.dma_start(out=wd_sb[:], in_=wd_t[:, :, hc * 128:(hc + 1) * 128])
                for oc2 in range(TOK // 512):
                    sl = slice(oc2 * 512, (oc2 + 1) * 512)
                    pn = psn.tile([128, 512], dt.float32)
                    for ff in range(NFT):
                        nc.tensor.matmul(pn[:], wd_sb[:, ff, :], aT[:, ff, sl],
                                         start=(ff == 0), stop=(ff == NFT - 1))
                    nc.vector.tensor_copy(dn_t[:, sl], pn[:])
                d = nc.sync.dma_start(
                    out=xd_d[bass.ds(off, 1), :, hc * TOK:(hc + 1) * TOK],
                    in_=dn_t[:])
                dwrites.append(d)

            b2 = sm4.tile([128, 1], dt.float32, tag="b2")
            nc.vector.memset(b2[:], 1.0)
            nc.sync.dma_start(out=b3i_d[:], in_=b2[:])
            cc2a = nc.gpsimd.collective_compute(
                "AllReduce", mybir.AluOpType.add, replica_groups=PAIRS,
                ins=[b3i_d[:].opt()], outs=[b3o_d[:].opt()])
            for d in dwrites[:NHT // 2]:
                tile.add_dep_helper(cc2a.ins, d.ins, sync=True, reason="dn writes before barrier")
            nc.sync.dma_start(out=b4i_d[:], in_=b2[:])
            cc2b = nc.gpsimd.collective_compute(
                "AllReduce", mybir.AluOpType.add, replica_groups=PAIRS,
                ins=[b4i_d[:].opt()], outs=[b4o_d[:].opt()])
            for d in dwrites[NHT // 2:]:
                tile.add_dep_helper(cc2b.ins, d.ins, sync=True, reason="dn writes before barrier")

        # ============ Phase 5: final residual + output (own token half only;
        # ============ the pair partner finalizes the other half)
        HT = TOK // 2
        with contextlib.ExitStack() as ph:
            fin = ph.enter_context(tc.tile_pool(name="fin", bufs=4))
            for hc in range(NHT):
                da_t = fin.tile([128, HT], dt.float32, tag="da")
                db_t = fin.tile([128, HT], dt.float32, tag="db")
                cc2 = cc2a if hc < NHT // 2 else cc2b
                da = nc.sync.dma_start(out=da_t[:],
                                       in_=xd_d[0, :, bass.ds(hc * TOK + off * HT, HT)])
                db = nc.sync.dma_start(out=db_t[:],
                                       in_=xd_d[1, :, bass.ds(hc * TOK + off * HT, HT)])
                tile.add_dep_helper(da.ins, cc2.ins, sync=True, reason="read after barrier2")
                tile.add_dep_helper(db.ins, cc2.ins, sync=True, reason="read after barrier2")
                xz = fin.tile([128, HT], dt.float32, tag="xz5")
                nc.sync.dma_start(out=xz[:], in_=x2_dt[:, hc, bass.ds(off * HT, HT)])
                f_t = fin.tile([128, HT], dt.float32, tag="f5")
                nc.vector.tensor_add(f_t[:], da_t[:], db_t[:])
                nc.vector.tensor_add(f_t[:], f_t[:], xz[:])
                nc.sync.dma_start(out=out_t[:, hc, bass.ds(off * HT, HT)], in_=f_t[:])

    return nc


_NC_CACHE = None


def _get_nc():
    global _NC_CACHE
    if _NC_CACHE is None:
        _NC_CACHE = build_nc()
        if not _NC_CACHE.is_finalized():
            _NC_CACHE.finalize()
    return _NC_CACHE


def make_in_maps(inputs):
    hs = np.asarray(inputs["hidden_states"], dtype=np.float32)
    mask = np.asarray(inputs["attention_mask"], dtype=np.float32)[0, 0]
    w = {k: np.asarray(inputs[k], dtype=np.float32) for k in
         ("w_q", "w_k", "w_v", "w_o", "w_gate", "w_up", "w_down")}
    g_in = np.asarray(inputs["g_in"], dtype=np.float32).reshape(H, 1)
    g_post = np.asarray(inputs["g_post"], dtype=np.float32).reshape(H, 1)
    bf = np.dtype("bfloat16") if hasattr(np, "bfloat16") else None
    import ml_dtypes
    bf16 = ml_dtypes.bfloat16

    in_maps = []
    for c in range(N_CORES):
        p, hh = c // 2, c % 2
        b, par = p // 2, p % 2
        xb = hs[b]                                    # [S, H]
        xt = np.ascontiguousarray(xb.T)               # [H, S]
        xt_own = np.ascontiguousarray(xb[par::2].T)   # [H, TOK]
        maskt = np.ascontiguousarray(mask[par::2].T)  # [S, TOK]
        cs = slice(hh * HH, (hh + 1) * HH)
        fs = slice(hh * FFH, (hh + 1) * FFH)
        in_maps.append({
            "xt": xt,
            "xt_own": xt_own,
            "maskt": maskt,
            "wq": np.ascontiguousarray(w["w_q"][:, cs]).astype(bf16),
            "wk": np.ascontiguousarray(w["w_k"][:, cs]).astype(bf16),
            "wv": np.ascontiguousarray(w["w_v"][:, cs]).astype(bf16),
            "wo": np.ascontiguousarray(w["w_o"][cs, :]).astype(bf16),
            "wg": np.ascontiguousarray(w["w_gate"][:, fs]).astype(bf16),
            "wu": np.ascontiguousarray(w["w_up"][:, fs]).astype(bf16),
            "wd": np.ascontiguousarray(w["w_down"][fs, :]).astype(bf16),
            "g_in": g_in,
            "g_post": g_post,
            "slot": np.array([[hh, 0]], dtype=np.uint32),
        })
    return in_maps


def assemble_output(results):
    out = np.empty((B, S, H), dtype=np.float32)
    ht = TOK // 2
    for b in range(B):
        for par in range(2):
            c = (2 * b + par) * 2
            pair_out = np.concatenate(
                [results[c]["out"][:, :ht], results[c + 1]["out"][:, ht:]], axis=1)
            out[b, par::2, :] = pair_out.T
    return out


def kernel(**inputs):
    nc = _get_nc()
    in_maps = make_in_maps(inputs)
    res = run_bass_kernel_spmd(nc, in_maps, list(range(N_CORES)))
    return assemble_output(res.results)


if __name__ == "__main__":
    import time
    t0 = time.time()
    nc = _get_nc()
    print(f"build+finalize: {time.time()-t0:.1f}s")
